# revision 2
# baseline (speedup 1.0000x reference)
"""Multi-head attention (B=2, S=4096, D=768, H=12) on 8 TRN2 NeuronCores. v5.

v3/v4 structure (fp8 DoubleRow P@V, exp split ScalarE/VectorE with a
Schraudolph bit-trick, deferred den normalize, 3-deep scores pipeline)
plus a unified PSUM pool: k/v/q projections allocate their accumulators
from the same [128, 1024] ring as the attention scores, so there are no
pool-scope barriers between phases, and the first attention unit's
groups are emitted interleaved with the projection stages - attention
starts ~3 projection stages into the kernel instead of after all of
them.
"""

import sys

sys.path.insert(0, "/opt/trn_rl_repo")

import numpy as np  # noqa: E402

from concourse import bacc, bass, mybir, tile  # noqa: E402
from concourse.bass_utils import run_bass_kernel_spmd  # noqa: E402

S = 4096
DM = 768
DK = 64
HPC = 3  # heads per core
NC_CORES = 8
KC = DM // 128  # 6 contraction chunks for projections
NSB = S // 512  # 8 seq blocks (projection N / attention q chunks)
NKV = S // 128  # 32 kv chunks
NG = NKV // 2  # 16 kv chunk-pairs (groups)
SCALE = 1.0 / np.sqrt(DK)
VPAD = 80  # v_aug plane stride (fp8 bytes), 16B-aligned

# Schraudolph exp for fp8e4m3 with round-to-nearest int conversion:
# bits = rint(8*log2(e)*x + 56 - 0.4634) ; bitcast int8 -> fp8e4m3
A8 = 8.0 * 1.4426950408889634
B8 = 56.0 - 0.4634
# group indices (of NG) handled by the DVE Schraudolph path; rest on ACT
DVE_GROUPS = frozenset({1, 3, 5, 7, 9, 12, 14})

F8 = mybir.dt.float8e4
F16 = mybir.dt.float16
F32 = mybir.dt.float32
I8 = mybir.dt.int8
DR = mybir.MatmulPerfMode.DoubleRow


def _emit(tc):
    nc = tc.nc
    qTx = nc.dram_tensor("qTx", [KC, NSB, 128, 512], F16, kind="ExternalInput").ap()
    kTx = nc.dram_tensor("kTx", [KC, NSB, 128, 512], F16, kind="ExternalInput").ap()
    vTx = nc.dram_tensor("vTx", [KC, NSB, 128, 512], F16, kind="ExternalInput").ap()
    wqT = nc.dram_tensor("wqT", [DM, HPC * DK], F16, kind="ExternalInput").ap()
    wkT = nc.dram_tensor("wkT", [DM, HPC * DK], F16, kind="ExternalInput").ap()
    wvT = nc.dram_tensor("wvT", [DM, HPC * DK], F16, kind="ExternalInput").ap()
    woT = nc.dram_tensor("woT", [HPC * DK, DM], F16, kind="ExternalInput").ap()
    bq = nc.dram_tensor("bq", [HPC * DK, 1], F32, kind="ExternalInput").ap()
    bk = nc.dram_tensor("bk", [HPC * DK, 1], F32, kind="ExternalInput").ap()
    bv = nc.dram_tensor("bv", [HPC * DK, 1], F32, kind="ExternalInput").ap()
    out_p = nc.dram_tensor("out_p", [S, DM], F16, kind="ExternalOutput").ap()
    den_d = nc.dram_tensor("den_d", [NSB * HPC, 512], F32, kind="Internal").ap()

    with (
        tc.tile_pool(name="const", bufs=1) as const,
        tc.tile_pool(name="heads", bufs=1) as heads,
        tc.tile_pool(name="xts", bufs=10) as xts,
        tc.tile_pool(name="work", bufs=3) as work,
        tc.tile_pool(name="norm", bufs=2) as norm,
        tc.tile_pool(name="sp", bufs=3, space=bass.MemorySpace.PSUM) as sp,
        tc.tile_pool(name="bigp", bufs=2, space=bass.MemorySpace.PSUM) as bigp,
    ):
        # ---- constants -------------------------------------------------
        w_q = const.tile([128, KC, HPC * DK], F16, tag="w_q")
        w_k = const.tile([128, KC, HPC * DK], F16, tag="w_k")
        w_v = const.tile([128, KC, HPC * DK], F16, tag="w_v")
        nc.sync.dma_start(w_k[:], wkT.rearrange("(c p) m -> p c m", p=128))
        nc.gpsimd.dma_start(w_v[:], wvT.rearrange("(c p) m -> p c m", p=128))
        wo01 = const.tile([128, DM], F16, tag="wo01")
        wo2 = const.tile([DK, DM], F16, tag="wo2")
        bq01 = const.tile([128, 1], F32, tag="bq01")
        bq2 = const.tile([DK, 1], F32, tag="bq2")
        bk01 = const.tile([128, 1], F32, tag="bk01")
        bk2 = const.tile([DK, 1], F32, tag="bk2")
        nc.sync.dma_start(bk01[:], bk[0:128, :])
        nc.sync.dma_start(bk2[:], bk[128:192, :])
        # v-bias broadcast to all 128 partitions: bvb[p, j] = bv[j]
        bvb = const.tile([128, HPC * DK], F32, tag="bvb")
        bv_bcast = bass.AP(
            tensor=bv.tensor, offset=bv.offset, ap=[[0, 128]] + list(bv.ap)
        )
        nc.gpsimd.dma_start(bvb[:], bv_bcast)
        nc.sync.dma_start(w_q[:], wqT.rearrange("(c p) m -> p c m", p=128))
        nc.sync.dma_start(bq01[:], bq[0:128, :])
        nc.sync.dma_start(bq2[:], bq[128:192, :])
        nc.gpsimd.dma_start(wo01[:], woT[0:128, :])
        nc.gpsimd.dma_start(wo2[:], woT[128:192, :])

        # preload the exp activation table right away
        warm = const.tile([1, 1], F32, tag="warm")
        nc.vector.memset(warm[:], 0.0)
        nc.scalar.activation(warm[:], warm[:], mybir.ActivationFunctionType.Exp)

        # ---- per-head persistent tensors ------------------------------
        qT2 = [heads.tile([128, S], F16, tag=f"qT2_{h}", name=f"qT2_{h}") for h in range(HPC)]
        kT2 = [heads.tile([128, S], F16, tag=f"kT2_{h}", name=f"kT2_{h}") for h in range(HPC)]
        # v_aug8: [128, NKV, VPAD] fp8; v_aug8[p, g, j] = v[kv=128g+p, dk j]
        # for j<64, col 64 = 1.0 (denominator), cols 65:80 pad (DR stride).
        v_aug8 = [
            heads.tile([128, NKV, VPAD], F8, tag=f"va8_{h}", name=f"va8_{h}")
            for h in range(HPC)
        ]
        for h in range(HPC):
            nc.vector.memset(v_aug8[h][:], 0.0)
            nc.vector.memset(v_aug8[h][:, :, 64:65], 1.0)
        ctx01 = heads.tile([128, S], F16, tag="ctx01")
        ctx2 = heads.tile([64, S], F16, tag="ctx2")

        # ---- projection helpers (accumulators from the shared sp ring) -
        def load_x_pair(dram, sbp, tag):
            # 6 tiles [128, 1024] fp16: dm-chunk kc x seq blocks 2sbp,2sbp+1
            xs = []
            for kc in range(KC):
                x2 = xts.tile([128, 1024], F16, tag=tag, bufs=10, name=f"{tag}_{sbp}_{kc}")
                eng = nc.sync if kc % 2 == 0 else nc.gpsimd
                eng.dma_start(
                    x2[:].rearrange("p (s n) -> p s n", s=2),
                    dram[kc, 2 * sbp : 2 * sbp + 2].rearrange("s p n -> p s n"),
                )
                xs.append(x2)
            return xs

        def kproj_half(sb, kxs, hsl):
            sq = bass.ts(sb, 512)
            kp = sp.tile([128, 1024], F32, tag="sT", name=f"kp_{sb}")
            for kc in range(KC):
                st = dict(start=(kc == 0), stop=(kc == KC - 1))
                nc.tensor.matmul(kp[:, 0:512], w_k[:, kc, 0:128], kxs[kc][:, hsl], **st)
                nc.tensor.matmul(kp[0:64, 512:1024], w_k[:, kc, 128:192], kxs[kc][:, hsl], **st)
            nc.vector.tensor_scalar_add(kT2[0][0:64, sq], kp[0:64, 0:512], bk01[0:64, :])
            nc.vector.tensor_scalar_add(kT2[1][0:64, sq], kp[64:128, 0:512], bk01[64:128, :])
            nc.vector.tensor_scalar_add(kT2[2][0:64, sq], kp[0:64, 512:1024], bk2[:])
            for h in range(HPC):
                nc.gpsimd.dma_start(kT2[h][64:128, sq], kT2[h][0:64, sq])

        def vproj_quad(base, vxs):
            # 4 kv chunks -> one sp tile; chunks 0,1 in bank A cols
            # 0:192/192:384, chunks 2,3 in bank B cols 512:704/704:896
            # (a matmul output may not cross a PSUM bank boundary).
            vp4 = sp.tile([128, 1024], F32, tag="sT", name=f"vp_{base}")
            cols = (0, 192, 512, 704)
            for c4 in range(4):
                ss = base % 8 + c4
                for kc in range(KC):
                    nc.tensor.matmul(
                        vp4[:, cols[c4] : cols[c4] + HPC * DK],
                        vxs[kc][:, bass.ts(ss, 128)],
                        w_v[:, kc, :],
                        start=(kc == 0),
                        stop=(kc == KC - 1),
                    )
            for c4 in range(4):
                g = base + c4
                for h in range(HPC):
                    nc.vector.tensor_add(
                        v_aug8[h][:, g, 0:64],
                        vp4[:, cols[c4] + h * DK : cols[c4] + (h + 1) * DK],
                        bvb[:, bass.ts(h, 64)],
                    )

        qp_state = {}

        def qproj_step(qc, kc):
            if kc == 0:
                qp_state[qc] = sp.tile([128, 1024], F32, tag="sT", name=f"qp_{qc}")
            qp = qp_state[qc]
            qx = xts.tile([128, 512], F16, tag="qx", bufs=6, name=f"qx_{qc}_{kc}")
            (nc.gpsimd if kc % 2 else nc.sync).dma_start(qx[:], qTx[kc, qc])
            st = dict(start=(kc == 0), stop=(kc == KC - 1))
            nc.tensor.matmul(qp[:, 0:512], w_q[:, kc, 0:128], qx[:], **st)
            nc.tensor.matmul(qp[0:64, 512:1024], w_q[:, kc, 128:192], qx[:], **st)

        def qproj_drain(qc):
            # drain + bias on ScalarE (Identity takes a per-partition bias
            # AP) so the DVE stays free for Schraudolph groups
            sq = bass.ts(qc, 512)
            qp = qp_state.pop(qc)
            Ident = mybir.ActivationFunctionType.Identity
            nc.scalar.activation(qT2[0][0:64, sq], qp[0:64, 0:512], Ident, bias=bq01[0:64, :])
            nc.scalar.activation(qT2[1][0:64, sq], qp[64:128, 0:512], Ident, bias=bq01[64:128, :])
            nc.scalar.activation(qT2[2][0:64, sq], qp[0:64, 512:1024], Ident, bias=bq2[:])
            for h in range(HPC):
                nc.gpsimd.dma_start(qT2[h][64:128, sq], qT2[h][0:64, sq])

        def qproj(qc):
            for kc in range(KC):
                qproj_step(qc, kc)
            qproj_drain(qc)

        # ---- output projection chain ----------------------------------
        def op_chain(qc, i):
            qs, half = i // 2, i % 2
            n0, nw = (0, 512) if half == 0 else (512, 256)
            qsl = bass.ds(qc * 512 + qs * 128, 128)
            op = bigp.tile([128, 512], F32, tag="big", name=f"op_{qc}_{i}")
            nc.tensor.matmul(
                op[:, 0:nw], ctx01[:, qsl], wo01[:, n0 : n0 + nw],
                start=True, stop=False,
            )
            nc.tensor.matmul(
                op[:, 0:nw], ctx2[:, qsl], wo2[:, n0 : n0 + nw],
                start=False, stop=True,
            )
            ob = work.tile([128, 512], F16, tag="ob", name=f"ob_{qc}_{i}")
            nc.vector.tensor_copy(ob[:, 0:nw], op[:, 0:nw])
            (nc.gpsimd if i % 2 else nc.sync).dma_start(
                out_p[qsl, n0 : n0 + nw], ob[:, 0:nw]
            )

        # ---- deferred den normalize ------------------------------------
        # unit-end ctx handling is double-deferred: at the NEXT unit's
        # gi==3 the ctx PSUM is staged to SBUF (2 ACT copies) + den DMA
        # round trip kicked off; at gi==6 the reciprocal+scale run on DVE.
        # This keeps waiting DR matmuls out of the PE queue at boundaries.
        norm_queue = []
        pending = []

        def norm_finish():
            if not pending:
                return
            stage, den, h, sq = pending.pop(0)
            rec = norm.tile([64, 512], F32, tag="rec")
            nc.vector.reciprocal_approx_fast(out=rec[:], in_=den[:])
            if h == 0:
                nc.vector.tensor_mul(ctx01[0:64, sq], stage[:], rec[:])
            elif h == 1:
                nc.vector.tensor_mul(ctx01[64:128, sq], stage[:], rec[:])
            else:
                nc.vector.tensor_mul(ctx2[:, sq], stage[:], rec[:])

        def norm_start(ctx, qc, h, sq):
            den_row = norm.tile([1, 512], F32, tag="den_row")
            nc.scalar.activation(
                den_row[:], ctx[64:65, :], mybir.ActivationFunctionType.Copy
            )
            stage = norm.tile([64, 512], F32, tag="stage", bufs=3)
            nc.scalar.activation(
                stage[:], ctx[0:64, :], mybir.ActivationFunctionType.Copy
            )
            di = qc * HPC + h
            nc.gpsimd.dma_start(den_d[di : di + 1, :], den_row[:])
            den = norm.tile([64, 512], F32, tag="den", bufs=3)
            dsrc = den_d[di : di + 1, :]
            den_bcast = bass.AP(
                tensor=dsrc.tensor,
                offset=dsrc.offset,
                ap=[[0, 64]] + list(dsrc.ap[1:]),
            )
            nc.gpsimd.dma_start(den[:], den_bcast)
            pending.append((stage, den, h, sq))

        # ---- attention group ------------------------------------------
        dr_queue = []  # (v_aug plane AP, pt tile, start, stop, ctx AP)

        def dr_flush(n):
            while len(dr_queue) > n:
                vag, pt, st, stp, ctx = dr_queue.pop(0)
                nc.tensor.matmul(ctx, vag, pt[:], start=st, stop=stp, perf_mode=DR)

        def emit_group(qc, h, gi, ctx):
            sq = bass.ts(qc, 512)
            kv = 2 * gi
            sT = sp.tile([128, 1024], F32, tag="sT")
            for j in range(2):
                lo = 64 if j == 1 else 0
                nc.tensor.matmul(
                    sT[:, bass.ts(j, 512)],
                    kT2[h][lo : lo + 64, bass.ts(kv + j, 128)],
                    qT2[h][lo : lo + 64, sq],
                )
            pt = work.tile([128, 2, 512], F8, tag="pt", bufs=6)
            if gi in DVE_GROUPS:
                nc.vector.tensor_scalar(
                    pt[:].bitcast(I8), sT[:].rearrange("p (s n) -> p s n", s=2),
                    A8 * SCALE, B8,
                    mybir.AluOpType.mult, mybir.AluOpType.add,
                )
            else:
                nc.scalar.activation(
                    pt[:], sT[:].rearrange("p (s n) -> p s n", s=2),
                    mybir.ActivationFunctionType.Exp, scale=SCALE,
                )
            dr_queue.append(
                (v_aug8[h][:, kv : kv + 2, 0:65], pt, gi == 0, gi == NG - 1, ctx)
            )
            dr_flush(4)
            # at gi==3 the previous unit's last DR has just been flushed
            # (queue depth 4), so its ctx is complete and safe to stage
            if gi == 3:
                while norm_queue:
                    norm_start(*norm_queue.pop(0))
            if gi == 6:
                norm_finish()
            # previous q-chunk's output projection, spread across h0 steps
            # (after gi==6 so the previous q-chunk's h2 normalize is done)
            if h == 0 and qc > 0 and 7 <= gi <= 14:
                op_chain(qc - 1, gi - 7)
            # next q-chunk's projection, spread across h1 steps
            if h == 1 and qc + 1 < NSB:
                if gi < KC:
                    qproj_step(qc + 1, gi)
                elif gi == KC:
                    qproj_drain(qc + 1)

        # ---- emission: projections interleaved with attention unit 0 ---
        ctx_u0 = bigp.tile([128, 512], F32, tag="big", name="ctx_u0")
        for sbp in range(NSB // 2):
            kxs = load_x_pair(kTx, sbp, "kx")
            kproj_half(2 * sbp, kxs, bass.ts(0, 512))
            kproj_half(2 * sbp + 1, kxs, bass.ts(1, 512))
            vxs = load_x_pair(vTx, sbp, "vx")
            vproj_quad(8 * sbp, vxs)
            vproj_quad(8 * sbp + 4, vxs)
            if sbp == 0:
                qproj(0)
            else:
                for gi in range(4 * (sbp - 1), 4 * sbp):
                    emit_group(0, 0, gi, ctx_u0[0:65, :])
        for gi in range(12, NG):
            emit_group(0, 0, gi, ctx_u0[0:65, :])
        norm_queue.append((ctx_u0[0:65, :], 0, 0, bass.ts(0, 512)))

        # ---- remaining units ------------------------------------------
        for qc in range(NSB):
            for h in range(HPC):
                if qc == 0 and h == 0:
                    continue
                ctx_t = bigp.tile([128, 512], F32, tag="big")
                ctx = ctx_t[0:65, :]
                for gi in range(NG):
                    emit_group(qc, h, gi, ctx)
                norm_queue.append((ctx, qc, h, bass.ts(qc, 512)))
        dr_flush(0)
        while norm_queue:
            norm_start(*norm_queue.pop(0))
            norm_finish()
        norm_finish()
        # last q-chunk's output projection
        for i in range(8):
            op_chain(NSB - 1, i)


_NC_CACHE = {}


def _build():
    if "nc" not in _NC_CACHE:
        nc = bacc.Bacc(
            "TRN2", target_bir_lowering=False, debug=False, num_devices=NC_CORES
        )
        with tile.TileContext(nc) as tc:
            _emit(tc)
        nc.compile()
        _NC_CACHE["nc"] = nc
    return _NC_CACHE["nc"]


def _tile_xT(x):
    # x: [S, DM] fp32 -> x.T tiled as [KC, NSB, 128, 512] fp16 so each
    # (kc, sb) DMA slice is one contiguous 128 KiB block
    xT = np.ascontiguousarray(x.T).astype(np.float16)  # [DM, S]
    t = xT.reshape(KC, 128, NSB, 512).transpose(0, 2, 1, 3)
    return np.ascontiguousarray(t)


def make_in_maps(query, key, value, wq, bq, wk, bk, wv, bv, wo, bo):
    query = np.asarray(query)
    key = np.asarray(key)
    value = np.asarray(value)
    wq, bq, wk, bk, wv, bv, wo, bo = (
        np.asarray(a) for a in (wq, bq, wk, bk, wv, bv, wo, bo)
    )
    in_maps = []
    for c in range(NC_CORES):
        b = c // 4
        hs = (c % 4) * HPC * DK
        he = hs + HPC * DK
        in_maps.append(
            {
                "qTx": _tile_xT(query[b]),
                "kTx": _tile_xT(key[b]),
                "vTx": _tile_xT(value[b]),
                "wqT": np.ascontiguousarray(wq[hs:he, :].T).astype(np.float16),
                "wkT": np.ascontiguousarray(wk[hs:he, :].T).astype(np.float16),
                "wvT": np.ascontiguousarray(wv[hs:he, :].T).astype(np.float16),
                "woT": np.ascontiguousarray(wo[:, hs:he].T).astype(np.float16),
                "bq": bq[hs:he].reshape(-1, 1).astype(np.float32),
                "bk": bk[hs:he].reshape(-1, 1).astype(np.float32),
                "bv": bv[hs:he].reshape(-1, 1).astype(np.float32),
            }
        )
    return in_maps


def combine_outputs(results, bo):
    parts = [results[c]["out_p"].astype(np.float32) for c in range(NC_CORES)]
    out0 = parts[0] + parts[1] + parts[2] + parts[3]
    out1 = parts[4] + parts[5] + parts[6] + parts[7]
    out = np.stack([out0, out1]) + np.asarray(bo)[None, None, :]
    return out.astype(np.float32)


def run_on_hw(in_maps, **kw):
    nc = _build()
    return run_bass_kernel_spmd(nc, in_maps, list(range(NC_CORES)), **kw)


def kernel(query, key, value, wq, bq, wk, bk, wv, bv, wo, bo):
    in_maps = make_in_maps(query, key, value, wq, bq, wk, bk, wv, bv, wo, bo)
    res = run_on_hw(in_maps)
    return combine_outputs(res.results, bo)
